# revision 35
# baseline (speedup 1.0000x reference)
"""Trainium2 Bass kernel for the CGN message-passing problem.

Math (per batch element b, node j of 20):
  xl_j = x[b, {j-1, j, j+1} mod 20]                     (3 values)
  o_j  = MLP(xl_j)  where MLP = combined block-diagonal MLP1||MLP2
         layers: 3 -> 32 -> 64 -> 32 -> 205 (relu between, last linear)
  f1[b, j]            = o_j[0]
  g1[b, j, band_j]    = o_j[1:19]     band_j = (6(j-1) + t) % 120, t<18
  f2[b, 6j:6j+6]      = o_j[19:25]
  g2[b, 6j+m, band'_j] = o_j[25+30m+t]  band'_j = (6(j-2) + t) % 120, t<30

Sharding: pure data parallel, batch 4096 -> 8 cores x 512 rows.
Matmul operands are bf16 (fp32 PSUM accumulation); outputs stay fp32.
Outputs are assembled densely in SBUF (zeros memset once per buffer; band
regions are rewritten every reuse) and stored with large contiguous DMAs.
"""

import ml_dtypes
import numpy as np

import concourse.bacc as bacc
import concourse.bass as bass
import concourse.tile as tile
from concourse import mybir
from concourse.bass_utils import run_bass_kernel_spmd

FP = mybir.dt.float32
BF = mybir.dt.bfloat16
NPBF = ml_dtypes.bfloat16
N_CORES = 8
B_FULL = 4096
BS = B_FULL // N_CORES      # 512 batch rows per core
P = 128                     # partition tile (batch sub-tile)
NCT = BS // P               # 4 batch sub-tiles per core
U = 20                      # nodes
GW = 120                    # dz * dim_u2
OD = 205                    # combined last-layer width (19 + 186)


def _g1_segs(j):
    # (dest_col_in_row, src_t, length); value t goes to col (6(j-1)+t) % 120
    s0 = (6 * (j - 1)) % GW
    n1 = min(18, GW - s0)
    segs = [(s0, 0, n1)]
    if n1 < 18:
        segs.append((0, n1, 18 - n1))
    return segs


def _g2_segs(j):
    # (dest_col_in_row, src_t, length); value (m, t) goes to col (6(j-2)+t) % 120
    s0 = (6 * (j - 2)) % GW
    n1 = min(30, GW - s0)
    segs = [(s0, 0, n1)]
    if n1 < 30:
        segs.append((0, n1, 30 - n1))
    return segs


def _build_bass():
    nc = bacc.Bacc("TRN2", target_bir_lowering=False)

    xt_d = nc.declare_dram_parameter("xt", [U, BS], BF, isOutput=False)
    wexp_d = nc.declare_dram_parameter("wexp", [U, U * 32], BF, isOutput=False)
    ba4_d = nc.declare_dram_parameter("ba4", [128, 1], FP, isOutput=False)
    wb4_d = nc.declare_dram_parameter("wb4", [128, 64], BF, isOutput=False)
    bb_d = nc.declare_dram_parameter("bb", [64, 1], FP, isOutput=False)
    wc_d = nc.declare_dram_parameter("wc", [64, 32], BF, isOutput=False)
    bc_d = nc.declare_dram_parameter("bc", [32, 1], FP, isOutput=False)
    wd_d = nc.declare_dram_parameter("wd", [33, OD], BF, isOutput=False)

    f1_d = nc.declare_dram_parameter("f1", [BS, U], FP, isOutput=True)
    g1_d = nc.declare_dram_parameter("g1", [BS, U * GW], FP, isOutput=True)
    f2_d = nc.declare_dram_parameter("f2", [BS, GW], FP, isOutput=True)
    g2_d = nc.declare_dram_parameter("g2", [BS, GW * GW], FP, isOutput=True)

    relu = mybir.ActivationFunctionType.Relu

    with tile.TileContext(nc) as tc:
        with (
            tc.tile_pool(name="consts", bufs=1) as consts,
            tc.tile_pool(name="h2", bufs=3) as h2p,
            tc.tile_pool(name="fout", bufs=2) as foutp,
            tc.tile_pool(name="psA", bufs=2, space="PSUM") as psAp,
            tc.tile_pool(name="ps2", bufs=2, space="PSUM") as ps2p,
            tc.tile_pool(name="ps3", bufs=2, space="PSUM") as ps3p,
            tc.tile_pool(name="ps4z", bufs=2, space="PSUM") as ps4p,
        ):
            # h3 tiles carry a ones row for the fused output bias; set those
            # first so layer-4 matmuls aren't queued behind big memsets.
            h3s = [consts.tile([33, P], BF, name=f"h3s{i}", tag=f"h3s{i}")
                   for i in range(4)]
            for t in h3s:
                nc.vector.memset(t[32:33, :], 1.0)

            # output assembly buffers: zeros persist, band cols rewritten.
            # memset region-by-region in consumption order, split across
            # engines, so the first flushes aren't gated on full-buffer zeroing.
            g2b = [consts.tile([128, GW * GW], FP, name=f"g2b{i}", tag=f"g2b{i}")
                   for i in range(2)]
            g1b = [consts.tile([128, U * GW], FP, name=f"g1b{i}", tag=f"g1b{i}")
                   for i in range(2)]
            q2 = GW * GW // 4
            q1 = U * GW // 4
            for jg in range(4):
                eng = nc.vector if jg < 2 else nc.gpsimd
                eng.memset(g2b[0][:, jg * q2:(jg + 1) * q2], 0.0)
                eng.memset(g1b[0][:, jg * q1:(jg + 1) * q1], 0.0)
            for jg in range(4):
                nc.gpsimd.memset(g2b[1][:, jg * q2:(jg + 1) * q2], 0.0)
                nc.gpsimd.memset(g1b[1][:, jg * q1:(jg + 1) * q1], 0.0)

            # ---- constants / weights (xT first: it gates layer 1) ----
            xT = consts.tile([U, BS], BF, tag="xT")
            nc.sync.dma_start(out=xT[:], in_=xt_d[:])
            wexp_t = consts.tile([U, U * 32], BF, tag="wexp")
            nc.sync.dma_start(out=wexp_t[:], in_=wexp_d[:])
            ba4_t = consts.tile([128, 1], FP, tag="ba4")
            nc.sync.dma_start(out=ba4_t[:], in_=ba4_d[:])
            wb4_t = consts.tile([128, 64], BF, tag="wb4")
            nc.sync.dma_start(out=wb4_t[:], in_=wb4_d[:])
            bb_t = consts.tile([64, 1], FP, tag="bb")
            nc.sync.dma_start(out=bb_t[:], in_=bb_d[:])
            wc_t = consts.tile([64, 32], BF, tag="wc")
            nc.sync.dma_start(out=wc_t[:], in_=wc_d[:])
            bc_t = consts.tile([32, 1], FP, tag="bc")
            nc.sync.dma_start(out=bc_t[:], in_=bc_d[:])
            # wd_t row 32 holds the output bias (ones row lives in h3s tiles)
            wd_t = consts.tile([33, OD], BF, tag="wd")
            nc.sync.dma_start(out=wd_t[:], in_=wd_d[:])

            # ---- layer 1 for all nodes: h1[m] holds nodes 4m..4m+3 ----
            h1 = []
            for m in range(5):
                psA = psAp.tile([128, BS], FP, tag="psA")
                nc.tensor.matmul(
                    psA[:], wexp_t[:, m * 128:(m + 1) * 128], xT[:],
                    start=True, stop=True,
                )
                h1m = consts.tile([128, BS], BF, name=f"h1_{m}", tag=f"h1_{m}")
                nc.scalar.activation(
                    out=h1m[:], in_=psA[:], func=relu, bias=ba4_t[:], scale=1.0
                )
                h1.append(h1m)

            # ---- layers 2-4 + banded assembly, per batch sub-tile ----
            # g2/g1 rows for node j live in a contiguous free-dim region, so
            # flush every 5 nodes to start output DMAs early.
            JG = 5
            for c in range(NCT):
                g2t = g2b[c % 2]
                g1t = g1b[c % 2]
                f1t = foutp.tile([128, U], FP, tag="f1t")
                f2t = foutp.tile([128, GW], FP, tag="f2t")
                g1v = g1t.rearrange("p (r q) -> p r q", q=GW)
                g2v = g2t.rearrange("p (r q) -> p r q", q=GW)
                cs = slice(c * P, (c + 1) * P)
                for j in range(U):
                    mm, k = j // 4, j % 4
                    ps2 = ps2p.tile([64, P], FP, tag="ps2")
                    nc.tensor.matmul(
                        ps2[:], wb4_t[32 * k:32 * k + 32, :],
                        h1[mm][32 * k:32 * k + 32, cs],
                        start=True, stop=True,
                        tile_position=(32 * k, 0),
                    )
                    h2 = h2p.tile([64, P], BF, tag="h2")
                    nc.scalar.activation(
                        out=h2[:], in_=ps2[:], func=relu, bias=bb_t[:], scale=1.0
                    )
                    ps3 = ps3p.tile([32, P], FP, tag="ps3")
                    nc.tensor.matmul(ps3[:], wc_t[:], h2[:], start=True, stop=True)
                    h3 = h3s[j % 4]
                    nc.scalar.activation(
                        out=h3[0:32, :], in_=ps3[:], func=relu, bias=bc_t[:], scale=1.0
                    )
                    ps4 = ps4p.tile([P, OD], FP, tag="ps4")
                    nc.tensor.matmul(ps4[:], h3[:], wd_t[:], start=True, stop=True)

                    nc.vector.tensor_copy(out=f1t[:, j:j + 1], in_=ps4[:, 0:1])
                    nc.vector.tensor_copy(out=f2t[:, 6 * j:6 * j + 6], in_=ps4[:, 19:25])
                    for (d0, t0, n) in _g1_segs(j):
                        nc.vector.tensor_copy(
                            out=g1v[:, j, d0:d0 + n], in_=ps4[:, 1 + t0:1 + t0 + n]
                        )
                    ps4g2 = ps4[:, 25:OD].rearrange("p (m t) -> p m t", t=30)
                    for (d0, t0, n) in _g2_segs(j):
                        nc.vector.tensor_copy(
                            out=g2v[:, 6 * j:6 * j + 6, d0:d0 + n],
                            in_=ps4g2[:, :, t0:t0 + n],
                        )
                    if j % JG == JG - 1:
                        jg = j // JG
                        r2 = slice(jg * JG * 6 * GW, (jg + 1) * JG * 6 * GW)
                        r1 = slice(jg * JG * GW, (jg + 1) * JG * GW)
                        nc.sync.dma_start(out=g2_d[cs, r2], in_=g2t[:, r2])
                        nc.sync.dma_start(out=g1_d[cs, r1], in_=g1t[:, r1])

                nc.sync.dma_start(out=f1_d[cs, :], in_=f1t[:])
                nc.sync.dma_start(out=f2_d[cs, :], in_=f2t[:])

    nc.compile()
    return nc


_NC_CACHE = None


def _get_nc():
    global _NC_CACHE
    if _NC_CACHE is None:
        _NC_CACHE = _build_bass()
    return _NC_CACHE


def _host_weights(inputs):
    def a(name):
        return np.ascontiguousarray(np.asarray(inputs[name]), dtype=np.float32)

    W1a, W2a = a("W1a"), a("W2a")
    b1a, b2a = a("b1a"), a("b2a")

    def blockdiag(A, B):
        o = np.zeros((A.shape[0] + B.shape[0], A.shape[1] + B.shape[1]), np.float32)
        o[:A.shape[0], :A.shape[1]] = A
        o[A.shape[0]:, A.shape[1]:] = B
        return o

    W_a = np.concatenate([W1a, W2a], axis=1)            # [3, 32]
    b_a = np.concatenate([b1a, b2a])                    # [32]
    W_b = blockdiag(a("W1b"), a("W2b"))                 # [32, 64]
    b_b = np.concatenate([a("b1b"), a("b2b")])          # [64]
    W_c = blockdiag(a("W1c"), a("W2c"))                 # [64, 32]
    b_c = np.concatenate([a("b1c"), a("b2c")])          # [32]
    W_d = blockdiag(a("W1d"), a("W2d"))                 # [32, 205]
    b_d = np.concatenate([a("b1d"), a("b2d")])          # [205]

    # expanded layer-1 stationary matrix: out block j = sum_r wexp[r, 32j:32j+32]*x[:, r]
    wexp = np.zeros((U, U * 32), np.float32)
    for j in range(U):
        wexp[(j - 1) % U, 32 * j:32 * j + 32] = W_a[0]
        wexp[j, 32 * j:32 * j + 32] = W_a[1]
        wexp[(j + 1) % U, 32 * j:32 * j + 32] = W_a[2]

    def bf(arr):
        return np.ascontiguousarray(arr.astype(NPBF))

    return {
        "wexp": bf(wexp),
        "ba4": np.ascontiguousarray(np.tile(b_a, 4)[:, None]),
        "wb4": bf(np.tile(W_b, (4, 1))),
        "bb": np.ascontiguousarray(b_b[:, None]),
        "wc": bf(W_c),
        "bc": np.ascontiguousarray(b_c[:, None]),
        "wd": bf(np.vstack([W_d, b_d[None, :]])),
    }


def make_in_maps(inputs):
    x = np.asarray(inputs["x"], dtype=np.float32)
    assert x.shape == (B_FULL, U), x.shape
    base = _host_weights(inputs)
    return [
        dict(base, xt=np.ascontiguousarray(x[i * BS:(i + 1) * BS].T.astype(NPBF)))
        for i in range(N_CORES)
    ]


def assemble_outputs(results):
    f1 = np.concatenate([r["f1"] for r in results], axis=0).reshape(B_FULL, U, 1)
    g1 = np.concatenate([r["g1"] for r in results], axis=0).reshape(B_FULL, U, GW)
    f2 = np.concatenate([r["f2"] for r in results], axis=0).reshape(B_FULL, GW, 1)
    g2 = np.concatenate([r["g2"] for r in results], axis=0).reshape(B_FULL, GW, GW)
    return f1, g1, f2, g2


def run_spmd(inputs, **kwargs):
    nc = _get_nc()
    in_maps = make_in_maps(inputs)
    return run_bass_kernel_spmd(nc, in_maps, list(range(N_CORES)), **kwargs)


def kernel(**inputs):
    # Retry once to ride out transient accelerator/runtime hiccups.
    try:
        res = run_spmd(inputs)
    except Exception:
        import time

        time.sleep(3.0)
        res = run_spmd(inputs)
    return assemble_outputs(res.results)


# revision 38
# speedup vs baseline: 1.0170x; 1.0170x over previous
"""Trainium2 Bass kernel for the CGN message-passing problem.

Math (per batch element b, node j of 20):
  xl_j = x[b, {j-1, j, j+1} mod 20]                     (3 values)
  o_j  = MLP(xl_j)  where MLP = combined block-diagonal MLP1||MLP2
         layers: 3 -> 32 -> 64 -> 32 -> 205 (relu between, last linear)
  f1[b, j]            = o_j[0]
  g1[b, j, band_j]    = o_j[1:19]     band_j = (6(j-1) + t) % 120, t<18
  f2[b, 6j:6j+6]      = o_j[19:25]
  g2[b, 6j+m, band'_j] = o_j[25+30m+t]  band'_j = (6(j-2) + t) % 120, t<30

Sharding: pure data parallel, batch 4096 -> 8 cores x 512 rows.
Matmul operands are bf16 (fp32 PSUM accumulation); outputs stay fp32.
Outputs are assembled densely in SBUF (zeros memset once per buffer; band
regions are rewritten every reuse) and stored with large contiguous DMAs.
"""

import ml_dtypes
import numpy as np

import concourse.bacc as bacc
import concourse.bass as bass
import concourse.tile as tile
from concourse import mybir
from concourse.bass_utils import run_bass_kernel_spmd

FP = mybir.dt.float32
BF = mybir.dt.bfloat16
NPBF = ml_dtypes.bfloat16
N_CORES = 8
B_FULL = 4096
BS = B_FULL // N_CORES      # 512 batch rows per core
P = 128                     # partition tile (batch sub-tile)
NCT = BS // P               # 4 batch sub-tiles per core
U = 20                      # nodes
GW = 120                    # dz * dim_u2
OD = 205                    # combined last-layer width (19 + 186)


def _g1_segs(j):
    # (dest_col_in_row, src_t, length); value t goes to col (6(j-1)+t) % 120
    s0 = (6 * (j - 1)) % GW
    n1 = min(18, GW - s0)
    segs = [(s0, 0, n1)]
    if n1 < 18:
        segs.append((0, n1, 18 - n1))
    return segs


def _g2_segs(j):
    # (dest_col_in_row, src_t, length); value (m, t) goes to col (6(j-2)+t) % 120
    s0 = (6 * (j - 2)) % GW
    n1 = min(30, GW - s0)
    segs = [(s0, 0, n1)]
    if n1 < 30:
        segs.append((0, n1, 30 - n1))
    return segs


def _build_bass():
    nc = bacc.Bacc("TRN2", target_bir_lowering=False)

    xt_d = nc.declare_dram_parameter("xt", [U, BS], BF, isOutput=False)
    wexp_d = nc.declare_dram_parameter("wexp", [U, U * 32], BF, isOutput=False)
    ba4_d = nc.declare_dram_parameter("ba4", [128, 1], FP, isOutput=False)
    wb4_d = nc.declare_dram_parameter("wb4", [128, 64], BF, isOutput=False)
    bb_d = nc.declare_dram_parameter("bb", [64, 1], FP, isOutput=False)
    wc_d = nc.declare_dram_parameter("wc", [64, 32], BF, isOutput=False)
    bc_d = nc.declare_dram_parameter("bc", [32, 1], FP, isOutput=False)
    wd_d = nc.declare_dram_parameter("wd", [33, OD], BF, isOutput=False)

    # f1 and f2 are packed into one tensor (cols 0:20 / 20:140) so the store
    # DMA writes 560B contiguous runs instead of sub-512B ones.
    ff_d = nc.declare_dram_parameter("ff", [BS, U + GW], FP, isOutput=True)
    g1_d = nc.declare_dram_parameter("g1", [BS, U * GW], FP, isOutput=True)
    g2_d = nc.declare_dram_parameter("g2", [BS, GW * GW], FP, isOutput=True)

    relu = mybir.ActivationFunctionType.Relu

    with tile.TileContext(nc) as tc:
        with (
            tc.tile_pool(name="consts", bufs=1) as consts,
            tc.tile_pool(name="h2", bufs=3) as h2p,
            tc.tile_pool(name="fout", bufs=2) as foutp,
            tc.tile_pool(name="psA", bufs=2, space="PSUM") as psAp,
            tc.tile_pool(name="ps2", bufs=2, space="PSUM") as ps2p,
            tc.tile_pool(name="ps3", bufs=2, space="PSUM") as ps3p,
            tc.tile_pool(name="ps4z", bufs=2, space="PSUM") as ps4p,
        ):
            # h3 tiles carry a ones row for the fused output bias; set those
            # first so layer-4 matmuls aren't queued behind big memsets.
            h3s = [consts.tile([33, P], BF, name=f"h3s{i}", tag=f"h3s{i}")
                   for i in range(4)]
            for t in h3s:
                nc.vector.memset(t[32:33, :], 1.0)

            # output assembly buffers: zeros persist, band cols rewritten.
            # memset region-by-region in consumption order, split across
            # engines, so the first flushes aren't gated on full-buffer zeroing.
            g2b = [consts.tile([128, GW * GW], FP, name=f"g2b{i}", tag=f"g2b{i}")
                   for i in range(2)]
            g1b = [consts.tile([128, U * GW], FP, name=f"g1b{i}", tag=f"g1b{i}")
                   for i in range(2)]
            q2 = GW * GW // 4
            q1 = U * GW // 4
            for jg in range(4):
                eng = nc.vector if jg < 2 else nc.gpsimd
                eng.memset(g2b[0][:, jg * q2:(jg + 1) * q2], 0.0)
                eng.memset(g1b[0][:, jg * q1:(jg + 1) * q1], 0.0)
            for jg in range(4):
                nc.gpsimd.memset(g2b[1][:, jg * q2:(jg + 1) * q2], 0.0)
                nc.gpsimd.memset(g1b[1][:, jg * q1:(jg + 1) * q1], 0.0)

            # ---- constants / weights (xT first: it gates layer 1) ----
            xT = consts.tile([U, BS], BF, tag="xT")
            nc.sync.dma_start(out=xT[:], in_=xt_d[:])
            wexp_t = consts.tile([U, U * 32], BF, tag="wexp")
            nc.sync.dma_start(out=wexp_t[:], in_=wexp_d[:])
            ba4_t = consts.tile([128, 1], FP, tag="ba4")
            nc.sync.dma_start(out=ba4_t[:], in_=ba4_d[:])
            wb4_t = consts.tile([128, 64], BF, tag="wb4")
            nc.sync.dma_start(out=wb4_t[:], in_=wb4_d[:])
            bb_t = consts.tile([64, 1], FP, tag="bb")
            nc.sync.dma_start(out=bb_t[:], in_=bb_d[:])
            wc_t = consts.tile([64, 32], BF, tag="wc")
            nc.sync.dma_start(out=wc_t[:], in_=wc_d[:])
            bc_t = consts.tile([32, 1], FP, tag="bc")
            nc.sync.dma_start(out=bc_t[:], in_=bc_d[:])
            # wd_t row 32 holds the output bias (ones row lives in h3s tiles)
            wd_t = consts.tile([33, OD], BF, tag="wd")
            nc.sync.dma_start(out=wd_t[:], in_=wd_d[:])

            # ---- layer 1 for all nodes: h1[m] holds nodes 4m..4m+3 ----
            h1 = []
            for m in range(5):
                psA = psAp.tile([128, BS], FP, tag="psA")
                nc.tensor.matmul(
                    psA[:], wexp_t[:, m * 128:(m + 1) * 128], xT[:],
                    start=True, stop=True,
                )
                h1m = consts.tile([128, BS], BF, name=f"h1_{m}", tag=f"h1_{m}")
                nc.scalar.activation(
                    out=h1m[:], in_=psA[:], func=relu, bias=ba4_t[:], scale=1.0
                )
                h1.append(h1m)

            # ---- layers 2-4 + banded assembly, per batch sub-tile ----
            # g2/g1 rows for node j live in a contiguous free-dim region.
            # c=0 flushes per-node for the first nodes (start the output
            # stream ASAP), then in larger chunks for DMA efficiency.
            for c in range(NCT):
                g2t = g2b[c % 2]
                g1t = g1b[c % 2]
                fft = foutp.tile([128, U + GW], FP, tag="fft")
                g1v = g1t.rearrange("p (r q) -> p r q", q=GW)
                g2v = g2t.rearrange("p (r q) -> p r q", q=GW)
                cs = slice(c * P, (c + 1) * P)
                flush_after = [0, 1, 2, 3, 4, 9, 14, 19] if c == 0 else [9, 19]
                prev_flush = -1
                for j in range(U):
                    mm, k = j // 4, j % 4
                    ps2 = ps2p.tile([64, P], FP, tag="ps2")
                    nc.tensor.matmul(
                        ps2[:], wb4_t[32 * k:32 * k + 32, :],
                        h1[mm][32 * k:32 * k + 32, cs],
                        start=True, stop=True,
                        tile_position=(32 * k, 0),
                    )
                    h2 = h2p.tile([64, P], BF, tag="h2")
                    nc.scalar.activation(
                        out=h2[:], in_=ps2[:], func=relu, bias=bb_t[:], scale=1.0
                    )
                    ps3 = ps3p.tile([32, P], FP, tag="ps3")
                    nc.tensor.matmul(ps3[:], wc_t[:], h2[:], start=True, stop=True)
                    h3 = h3s[j % 4]
                    nc.scalar.activation(
                        out=h3[0:32, :], in_=ps3[:], func=relu, bias=bc_t[:], scale=1.0
                    )
                    ps4 = ps4p.tile([P, OD], FP, tag="ps4")
                    nc.tensor.matmul(ps4[:], h3[:], wd_t[:], start=True, stop=True)

                    ps4g2 = ps4[:, 25:OD].rearrange("p (m t) -> p m t", t=30)
                    for (d0, t0, n) in _g2_segs(j):
                        nc.vector.tensor_copy(
                            out=g2v[:, 6 * j:6 * j + 6, d0:d0 + n],
                            in_=ps4g2[:, :, t0:t0 + n],
                        )
                    for (d0, t0, n) in _g1_segs(j):
                        nc.vector.tensor_copy(
                            out=g1v[:, j, d0:d0 + n], in_=ps4[:, 1 + t0:1 + t0 + n]
                        )
                    nc.vector.tensor_copy(out=fft[:, j:j + 1], in_=ps4[:, 0:1])
                    nc.vector.tensor_copy(
                        out=fft[:, U + 6 * j:U + 6 * j + 6], in_=ps4[:, 19:25]
                    )
                    if j in flush_after:
                        r2 = slice((prev_flush + 1) * 6 * GW, (j + 1) * 6 * GW)
                        r1 = slice((prev_flush + 1) * GW, (j + 1) * GW)
                        nc.sync.dma_start(out=g2_d[cs, r2], in_=g2t[:, r2])
                        nc.sync.dma_start(out=g1_d[cs, r1], in_=g1t[:, r1])
                        prev_flush = j

                nc.sync.dma_start(out=ff_d[cs, :], in_=fft[:])

    nc.compile()
    return nc


_NC_CACHE = None


def _get_nc():
    global _NC_CACHE
    if _NC_CACHE is None:
        _NC_CACHE = _build_bass()
    return _NC_CACHE


def _host_weights(inputs):
    def a(name):
        return np.ascontiguousarray(np.asarray(inputs[name]), dtype=np.float32)

    W1a, W2a = a("W1a"), a("W2a")
    b1a, b2a = a("b1a"), a("b2a")

    def blockdiag(A, B):
        o = np.zeros((A.shape[0] + B.shape[0], A.shape[1] + B.shape[1]), np.float32)
        o[:A.shape[0], :A.shape[1]] = A
        o[A.shape[0]:, A.shape[1]:] = B
        return o

    W_a = np.concatenate([W1a, W2a], axis=1)            # [3, 32]
    b_a = np.concatenate([b1a, b2a])                    # [32]
    W_b = blockdiag(a("W1b"), a("W2b"))                 # [32, 64]
    b_b = np.concatenate([a("b1b"), a("b2b")])          # [64]
    W_c = blockdiag(a("W1c"), a("W2c"))                 # [64, 32]
    b_c = np.concatenate([a("b1c"), a("b2c")])          # [32]
    W_d = blockdiag(a("W1d"), a("W2d"))                 # [32, 205]
    b_d = np.concatenate([a("b1d"), a("b2d")])          # [205]

    # expanded layer-1 stationary matrix: out block j = sum_r wexp[r, 32j:32j+32]*x[:, r]
    wexp = np.zeros((U, U * 32), np.float32)
    for j in range(U):
        wexp[(j - 1) % U, 32 * j:32 * j + 32] = W_a[0]
        wexp[j, 32 * j:32 * j + 32] = W_a[1]
        wexp[(j + 1) % U, 32 * j:32 * j + 32] = W_a[2]

    def bf(arr):
        return np.ascontiguousarray(arr.astype(NPBF))

    return {
        "wexp": bf(wexp),
        "ba4": np.ascontiguousarray(np.tile(b_a, 4)[:, None]),
        "wb4": bf(np.tile(W_b, (4, 1))),
        "bb": np.ascontiguousarray(b_b[:, None]),
        "wc": bf(W_c),
        "bc": np.ascontiguousarray(b_c[:, None]),
        "wd": bf(np.vstack([W_d, b_d[None, :]])),
    }


def make_in_maps(inputs):
    x = np.asarray(inputs["x"], dtype=np.float32)
    assert x.shape == (B_FULL, U), x.shape
    base = _host_weights(inputs)
    return [
        dict(base, xt=np.ascontiguousarray(x[i * BS:(i + 1) * BS].T.astype(NPBF)))
        for i in range(N_CORES)
    ]


def assemble_outputs(results):
    ff = np.concatenate([r["ff"] for r in results], axis=0)
    f1 = np.ascontiguousarray(ff[:, :U]).reshape(B_FULL, U, 1)
    f2 = np.ascontiguousarray(ff[:, U:]).reshape(B_FULL, GW, 1)
    g1 = np.concatenate([r["g1"] for r in results], axis=0).reshape(B_FULL, U, GW)
    g2 = np.concatenate([r["g2"] for r in results], axis=0).reshape(B_FULL, GW, GW)
    return f1, g1, f2, g2


def run_spmd(inputs, **kwargs):
    nc = _get_nc()
    in_maps = make_in_maps(inputs)
    return run_bass_kernel_spmd(nc, in_maps, list(range(N_CORES)), **kwargs)


def kernel(**inputs):
    # Retry once to ride out transient accelerator/runtime hiccups.
    try:
        res = run_spmd(inputs)
    except Exception:
        import time

        time.sleep(3.0)
        res = run_spmd(inputs)
    return assemble_outputs(res.results)


# revision 39
# speedup vs baseline: 1.1650x; 1.1455x over previous
"""Trainium2 Bass kernel for the CGN message-passing problem.

Math (per batch element b, node j of 20):
  xl_j = x[b, {j-1, j, j+1} mod 20]                     (3 values)
  o_j  = MLP(xl_j)  where MLP = combined block-diagonal MLP1||MLP2
         layers: 3 -> 32 -> 64 -> 32 -> 205 (relu between, last linear)
  f1[b, j]            = o_j[0]
  g1[b, j, band_j]    = o_j[1:19]     band_j = (6(j-1) + t) % 120, t<18
  f2[b, 6j:6j+6]      = o_j[19:25]
  g2[b, 6j+m, band'_j] = o_j[25+30m+t]  band'_j = (6(j-2) + t) % 120, t<30

Sharding: pure data parallel, batch 4096 -> 8 cores x 512 rows.
Matmul operands are bf16 (fp32 PSUM accumulation); outputs stay fp32.
Outputs are assembled densely in SBUF (zeros memset once per buffer; band
regions are rewritten every reuse) and stored with large contiguous DMAs.
"""

import ml_dtypes
import numpy as np

import concourse.bacc as bacc
import concourse.bass as bass
import concourse.tile as tile
from concourse import mybir
from concourse.bass_utils import run_bass_kernel_spmd

FP = mybir.dt.float32
BF = mybir.dt.bfloat16
NPBF = ml_dtypes.bfloat16
N_CORES = 8
B_FULL = 4096
BS = B_FULL // N_CORES      # 512 batch rows per core
P = 128                     # partition tile (batch sub-tile)
NCT = BS // P               # 4 batch sub-tiles per core
U = 20                      # nodes
GW = 120                    # dz * dim_u2
OD = 205                    # combined last-layer width (19 + 186)


def _g1_segs(j):
    # (dest_col_in_row, src_t, length); value t goes to col (6(j-1)+t) % 120
    s0 = (6 * (j - 1)) % GW
    n1 = min(18, GW - s0)
    segs = [(s0, 0, n1)]
    if n1 < 18:
        segs.append((0, n1, 18 - n1))
    return segs


def _g2_segs(j):
    # (dest_col_in_row, src_t, length); value (m, t) goes to col (6(j-2)+t) % 120
    s0 = (6 * (j - 2)) % GW
    n1 = min(30, GW - s0)
    segs = [(s0, 0, n1)]
    if n1 < 30:
        segs.append((0, n1, 30 - n1))
    return segs


def _build_bass():
    nc = bacc.Bacc("TRN2", target_bir_lowering=False)

    xt_d = nc.declare_dram_parameter("xt", [U, BS], BF, isOutput=False)
    wexp_d = nc.declare_dram_parameter("wexp", [U, U * 32], BF, isOutput=False)
    ba4_d = nc.declare_dram_parameter("ba4", [128, 1], FP, isOutput=False)
    wb4_d = nc.declare_dram_parameter("wb4", [128, 64], BF, isOutput=False)
    bb_d = nc.declare_dram_parameter("bb", [64, 1], FP, isOutput=False)
    wc_d = nc.declare_dram_parameter("wc", [64, 32], BF, isOutput=False)
    bc_d = nc.declare_dram_parameter("bc", [32, 1], FP, isOutput=False)
    wd_d = nc.declare_dram_parameter("wd", [33, OD], BF, isOutput=False)

    # f1 and f2 are packed into one tensor (cols 0:20 / 20:140) so the store
    # DMA writes 560B contiguous runs instead of sub-512B ones.
    ff_d = nc.declare_dram_parameter("ff", [BS, U + GW], FP, isOutput=True)
    g1_d = nc.declare_dram_parameter("g1", [BS, U * GW], FP, isOutput=True)
    g2_d = nc.declare_dram_parameter("g2", [BS, GW * GW], FP, isOutput=True)

    relu = mybir.ActivationFunctionType.Relu

    with tile.TileContext(nc) as tc:
        with (
            tc.tile_pool(name="consts", bufs=1) as consts,
            tc.tile_pool(name="h2", bufs=3) as h2p,
            tc.tile_pool(name="fout", bufs=2) as foutp,
            tc.tile_pool(name="psA", bufs=2, space="PSUM") as psAp,
            tc.tile_pool(name="ps2", bufs=2, space="PSUM") as ps2p,
            tc.tile_pool(name="ps3", bufs=2, space="PSUM") as ps3p,
            tc.tile_pool(name="ps4z", bufs=2, space="PSUM") as ps4p,
        ):
            # h3 tiles carry a ones row for the fused output bias; set those
            # first so layer-4 matmuls aren't queued behind big memsets.
            h3s = [consts.tile([33, P], BF, name=f"h3s{i}", tag=f"h3s{i}")
                   for i in range(4)]
            for t in h3s:
                nc.vector.memset(t[32:33, :], 1.0)

            # output assembly buffers: zeros persist, band cols rewritten.
            # memset region-by-region in consumption order, split across
            # engines, so the first flushes aren't gated on full-buffer zeroing.
            g2b = [consts.tile([128, GW * GW], FP, name=f"g2b{i}", tag=f"g2b{i}")
                   for i in range(2)]
            g1b = [consts.tile([128, U * GW], FP, name=f"g1b{i}", tag=f"g1b{i}")
                   for i in range(2)]
            q2 = GW * GW // 4
            q1 = U * GW // 4
            for jg in range(4):
                eng = nc.vector if jg < 2 else nc.gpsimd
                eng.memset(g2b[0][:, jg * q2:(jg + 1) * q2], 0.0)
                eng.memset(g1b[0][:, jg * q1:(jg + 1) * q1], 0.0)
            for jg in range(4):
                nc.gpsimd.memset(g2b[1][:, jg * q2:(jg + 1) * q2], 0.0)
                nc.gpsimd.memset(g1b[1][:, jg * q1:(jg + 1) * q1], 0.0)

            # ---- constants / weights (xT first: it gates layer 1) ----
            xT = consts.tile([U, BS], BF, tag="xT")
            nc.sync.dma_start(out=xT[:], in_=xt_d[:])
            wexp_t = consts.tile([U, U * 32], BF, tag="wexp")
            nc.sync.dma_start(out=wexp_t[:], in_=wexp_d[:])
            ba4_t = consts.tile([128, 1], FP, tag="ba4")
            nc.sync.dma_start(out=ba4_t[:], in_=ba4_d[:])
            wb4_t = consts.tile([128, 64], BF, tag="wb4")
            nc.sync.dma_start(out=wb4_t[:], in_=wb4_d[:])
            bb_t = consts.tile([64, 1], FP, tag="bb")
            nc.sync.dma_start(out=bb_t[:], in_=bb_d[:])
            wc_t = consts.tile([64, 32], BF, tag="wc")
            nc.sync.dma_start(out=wc_t[:], in_=wc_d[:])
            bc_t = consts.tile([32, 1], FP, tag="bc")
            nc.sync.dma_start(out=bc_t[:], in_=bc_d[:])
            # wd_t row 32 holds the output bias (ones row lives in h3s tiles)
            wd_t = consts.tile([33, OD], BF, tag="wd")
            nc.sync.dma_start(out=wd_t[:], in_=wd_d[:])

            # ---- layer 1 for all nodes: h1[m] holds nodes 4m..4m+3 ----
            h1 = []
            for m in range(5):
                psA = psAp.tile([128, BS], FP, tag="psA")
                nc.tensor.matmul(
                    psA[:], wexp_t[:, m * 128:(m + 1) * 128], xT[:],
                    start=True, stop=True,
                )
                h1m = consts.tile([128, BS], BF, name=f"h1_{m}", tag=f"h1_{m}")
                nc.scalar.activation(
                    out=h1m[:], in_=psA[:], func=relu, bias=ba4_t[:], scale=1.0
                )
                h1.append(h1m)

            # ---- layers 2-4 + banded assembly, per batch sub-tile ----
            # g2/g1 rows for node j live in a contiguous free-dim region.
            # c=0 flushes per-node for the first nodes (start the output
            # stream ASAP), then in larger chunks for DMA efficiency.
            for c in range(NCT):
                g2t = g2b[c % 2]
                g1t = g1b[c % 2]
                fft = foutp.tile([128, U + GW], FP, tag="fft")
                g1v = g1t.rearrange("p (r q) -> p r q", q=GW)
                g2v = g2t.rearrange("p (r q) -> p r q", q=GW)
                cs = slice(c * P, (c + 1) * P)
                flush_after = ([0, 1, 2, 3, 4, 6, 8, 11, 14, 19] if c == 0
                               else [4, 9, 14, 19])
                prev_flush = -1
                for j in range(U):
                    mm, k = j // 4, j % 4
                    ps2 = ps2p.tile([64, P], FP, tag="ps2")
                    nc.tensor.matmul(
                        ps2[:], wb4_t[32 * k:32 * k + 32, :],
                        h1[mm][32 * k:32 * k + 32, cs],
                        start=True, stop=True,
                        tile_position=(32 * k, 0),
                    )
                    h2 = h2p.tile([64, P], BF, tag="h2")
                    nc.scalar.activation(
                        out=h2[:], in_=ps2[:], func=relu, bias=bb_t[:], scale=1.0
                    )
                    ps3 = ps3p.tile([32, P], FP, tag="ps3")
                    nc.tensor.matmul(ps3[:], wc_t[:], h2[:], start=True, stop=True)
                    h3 = h3s[j % 4]
                    nc.scalar.activation(
                        out=h3[0:32, :], in_=ps3[:], func=relu, bias=bc_t[:], scale=1.0
                    )
                    ps4 = ps4p.tile([P, OD], FP, tag="ps4")
                    nc.tensor.matmul(ps4[:], h3[:], wd_t[:], start=True, stop=True)

                    ps4g2 = ps4[:, 25:OD].rearrange("p (m t) -> p m t", t=30)
                    for (d0, t0, n) in _g2_segs(j):
                        nc.vector.tensor_copy(
                            out=g2v[:, 6 * j:6 * j + 6, d0:d0 + n],
                            in_=ps4g2[:, :, t0:t0 + n],
                        )
                    for (d0, t0, n) in _g1_segs(j):
                        nc.vector.tensor_copy(
                            out=g1v[:, j, d0:d0 + n], in_=ps4[:, 1 + t0:1 + t0 + n]
                        )
                    nc.vector.tensor_copy(out=fft[:, j:j + 1], in_=ps4[:, 0:1])
                    nc.vector.tensor_copy(
                        out=fft[:, U + 6 * j:U + 6 * j + 6], in_=ps4[:, 19:25]
                    )
                    if j in flush_after:
                        r2 = slice((prev_flush + 1) * 6 * GW, (j + 1) * 6 * GW)
                        r1 = slice((prev_flush + 1) * GW, (j + 1) * GW)
                        nc.sync.dma_start(out=g2_d[cs, r2], in_=g2t[:, r2])
                        nc.sync.dma_start(out=g1_d[cs, r1], in_=g1t[:, r1])
                        prev_flush = j

                nc.sync.dma_start(out=ff_d[cs, :], in_=fft[:])

    nc.compile()
    return nc


_NC_CACHE = None


def _get_nc():
    global _NC_CACHE
    if _NC_CACHE is None:
        _NC_CACHE = _build_bass()
    return _NC_CACHE


def _host_weights(inputs):
    def a(name):
        return np.ascontiguousarray(np.asarray(inputs[name]), dtype=np.float32)

    W1a, W2a = a("W1a"), a("W2a")
    b1a, b2a = a("b1a"), a("b2a")

    def blockdiag(A, B):
        o = np.zeros((A.shape[0] + B.shape[0], A.shape[1] + B.shape[1]), np.float32)
        o[:A.shape[0], :A.shape[1]] = A
        o[A.shape[0]:, A.shape[1]:] = B
        return o

    W_a = np.concatenate([W1a, W2a], axis=1)            # [3, 32]
    b_a = np.concatenate([b1a, b2a])                    # [32]
    W_b = blockdiag(a("W1b"), a("W2b"))                 # [32, 64]
    b_b = np.concatenate([a("b1b"), a("b2b")])          # [64]
    W_c = blockdiag(a("W1c"), a("W2c"))                 # [64, 32]
    b_c = np.concatenate([a("b1c"), a("b2c")])          # [32]
    W_d = blockdiag(a("W1d"), a("W2d"))                 # [32, 205]
    b_d = np.concatenate([a("b1d"), a("b2d")])          # [205]

    # expanded layer-1 stationary matrix: out block j = sum_r wexp[r, 32j:32j+32]*x[:, r]
    wexp = np.zeros((U, U * 32), np.float32)
    for j in range(U):
        wexp[(j - 1) % U, 32 * j:32 * j + 32] = W_a[0]
        wexp[j, 32 * j:32 * j + 32] = W_a[1]
        wexp[(j + 1) % U, 32 * j:32 * j + 32] = W_a[2]

    def bf(arr):
        return np.ascontiguousarray(arr.astype(NPBF))

    return {
        "wexp": bf(wexp),
        "ba4": np.ascontiguousarray(np.tile(b_a, 4)[:, None]),
        "wb4": bf(np.tile(W_b, (4, 1))),
        "bb": np.ascontiguousarray(b_b[:, None]),
        "wc": bf(W_c),
        "bc": np.ascontiguousarray(b_c[:, None]),
        "wd": bf(np.vstack([W_d, b_d[None, :]])),
    }


def make_in_maps(inputs):
    x = np.asarray(inputs["x"], dtype=np.float32)
    assert x.shape == (B_FULL, U), x.shape
    base = _host_weights(inputs)
    return [
        dict(base, xt=np.ascontiguousarray(x[i * BS:(i + 1) * BS].T.astype(NPBF)))
        for i in range(N_CORES)
    ]


def assemble_outputs(results):
    ff = np.concatenate([r["ff"] for r in results], axis=0)
    f1 = np.ascontiguousarray(ff[:, :U]).reshape(B_FULL, U, 1)
    f2 = np.ascontiguousarray(ff[:, U:]).reshape(B_FULL, GW, 1)
    g1 = np.concatenate([r["g1"] for r in results], axis=0).reshape(B_FULL, U, GW)
    g2 = np.concatenate([r["g2"] for r in results], axis=0).reshape(B_FULL, GW, GW)
    return f1, g1, f2, g2


def run_spmd(inputs, **kwargs):
    nc = _get_nc()
    in_maps = make_in_maps(inputs)
    return run_bass_kernel_spmd(nc, in_maps, list(range(N_CORES)), **kwargs)


def kernel(**inputs):
    # Retry once to ride out transient accelerator/runtime hiccups.
    try:
        res = run_spmd(inputs)
    except Exception:
        import time

        time.sleep(3.0)
        res = run_spmd(inputs)
    return assemble_outputs(res.results)


# revision 40
# speedup vs baseline: 1.1749x; 1.0085x over previous
"""Trainium2 Bass kernel for the CGN message-passing problem.

Math (per batch element b, node j of 20):
  xl_j = x[b, {j-1, j, j+1} mod 20]                     (3 values)
  o_j  = MLP(xl_j)  where MLP = combined block-diagonal MLP1||MLP2
         layers: 3 -> 32 -> 64 -> 32 -> 205 (relu between, last linear)
  f1[b, j]            = o_j[0]
  g1[b, j, band_j]    = o_j[1:19]     band_j = (6(j-1) + t) % 120, t<18
  f2[b, 6j:6j+6]      = o_j[19:25]
  g2[b, 6j+m, band'_j] = o_j[25+30m+t]  band'_j = (6(j-2) + t) % 120, t<30

Sharding: pure data parallel, batch 4096 -> 8 cores x 512 rows.
Matmul operands are bf16 (fp32 PSUM accumulation); outputs stay fp32.
Outputs are assembled densely in SBUF (zeros memset once per buffer; band
regions are rewritten every reuse) and stored with large contiguous DMAs.
"""

import ml_dtypes
import numpy as np

import concourse.bacc as bacc
import concourse.bass as bass
import concourse.tile as tile
from concourse import mybir
from concourse.bass_utils import run_bass_kernel_spmd

FP = mybir.dt.float32
BF = mybir.dt.bfloat16
NPBF = ml_dtypes.bfloat16
N_CORES = 8
B_FULL = 4096
BS = B_FULL // N_CORES      # 512 batch rows per core
P = 128                     # partition tile (batch sub-tile)
NCT = BS // P               # 4 batch sub-tiles per core
U = 20                      # nodes
GW = 120                    # dz * dim_u2
OD = 205                    # combined last-layer width (19 + 186)


def _g1_segs(j):
    # (dest_col_in_row, src_t, length); value t goes to col (6(j-1)+t) % 120
    s0 = (6 * (j - 1)) % GW
    n1 = min(18, GW - s0)
    segs = [(s0, 0, n1)]
    if n1 < 18:
        segs.append((0, n1, 18 - n1))
    return segs


def _g2_segs(j):
    # (dest_col_in_row, src_t, length); value (m, t) goes to col (6(j-2)+t) % 120
    s0 = (6 * (j - 2)) % GW
    n1 = min(30, GW - s0)
    segs = [(s0, 0, n1)]
    if n1 < 30:
        segs.append((0, n1, 30 - n1))
    return segs


def _build_bass():
    nc = bacc.Bacc("TRN2", target_bir_lowering=False)

    xt_d = nc.declare_dram_parameter("xt", [U, BS], BF, isOutput=False)
    wexp_d = nc.declare_dram_parameter("wexp", [U, U * 32], BF, isOutput=False)
    ba4_d = nc.declare_dram_parameter("ba4", [128, 1], FP, isOutput=False)
    wb4_d = nc.declare_dram_parameter("wb4", [128, 64], BF, isOutput=False)
    bb_d = nc.declare_dram_parameter("bb", [64, 1], FP, isOutput=False)
    wc_d = nc.declare_dram_parameter("wc", [64, 32], BF, isOutput=False)
    bc_d = nc.declare_dram_parameter("bc", [32, 1], FP, isOutput=False)
    wd_d = nc.declare_dram_parameter("wd", [33, OD], BF, isOutput=False)

    # f1 and f2 are packed into one tensor (cols 0:20 / 20:140) so the store
    # DMA writes 560B contiguous runs instead of sub-512B ones.
    ff_d = nc.declare_dram_parameter("ff", [BS, U + GW], FP, isOutput=True)
    g1_d = nc.declare_dram_parameter("g1", [BS, U * GW], FP, isOutput=True)
    g2_d = nc.declare_dram_parameter("g2", [BS, GW * GW], FP, isOutput=True)

    relu = mybir.ActivationFunctionType.Relu

    with tile.TileContext(nc) as tc:
        with (
            tc.tile_pool(name="consts", bufs=1) as consts,
            tc.tile_pool(name="h2", bufs=3) as h2p,
            tc.tile_pool(name="fout", bufs=2) as foutp,
            tc.tile_pool(name="psA", bufs=2, space="PSUM") as psAp,
            tc.tile_pool(name="ps2", bufs=2, space="PSUM") as ps2p,
            tc.tile_pool(name="ps3", bufs=2, space="PSUM") as ps3p,
            tc.tile_pool(name="ps4z", bufs=2, space="PSUM") as ps4p,
        ):
            # h3 tiles carry a ones row for the fused output bias; set those
            # first so layer-4 matmuls aren't queued behind big memsets.
            h3s = [consts.tile([33, P], BF, name=f"h3s{i}", tag=f"h3s{i}")
                   for i in range(4)]
            for t in h3s:
                nc.vector.memset(t[32:33, :], 1.0)

            # output assembly buffers: zeros persist, band cols rewritten.
            # memset region-by-region in consumption order, split across
            # engines, so the first flushes aren't gated on full-buffer zeroing.
            g2b = [consts.tile([128, GW * GW], FP, name=f"g2b{i}", tag=f"g2b{i}")
                   for i in range(2)]
            g1b = [consts.tile([128, U * GW], FP, name=f"g1b{i}", tag=f"g1b{i}")
                   for i in range(2)]
            q2 = GW * GW // 4
            q1 = U * GW // 4
            for jg in range(4):
                eng = nc.vector if jg < 2 else nc.gpsimd
                eng.memset(g2b[0][:, jg * q2:(jg + 1) * q2], 0.0)
                eng.memset(g1b[0][:, jg * q1:(jg + 1) * q1], 0.0)
            for jg in range(4):
                nc.gpsimd.memset(g2b[1][:, jg * q2:(jg + 1) * q2], 0.0)
                nc.gpsimd.memset(g1b[1][:, jg * q1:(jg + 1) * q1], 0.0)

            # ---- constants / weights, in need-order, split across the two
            # HWDGE rings (SP + ACT) so the tiny loads land in parallel ----
            xT = consts.tile([U, BS], BF, tag="xT")
            nc.sync.dma_start(out=xT[:], in_=xt_d[:])
            wexp_t = consts.tile([U, U * 32], BF, tag="wexp")
            nc.scalar.dma_start(out=wexp_t[:], in_=wexp_d[:])
            ba4_t = consts.tile([128, 1], FP, tag="ba4")
            nc.sync.dma_start(out=ba4_t[:], in_=ba4_d[:])
            wb4_t = consts.tile([128, 64], BF, tag="wb4")
            nc.scalar.dma_start(out=wb4_t[:], in_=wb4_d[:])
            bb_t = consts.tile([64, 1], FP, tag="bb")
            nc.sync.dma_start(out=bb_t[:], in_=bb_d[:])
            wc_t = consts.tile([64, 32], BF, tag="wc")
            nc.scalar.dma_start(out=wc_t[:], in_=wc_d[:])
            bc_t = consts.tile([32, 1], FP, tag="bc")
            nc.sync.dma_start(out=bc_t[:], in_=bc_d[:])
            # wd_t row 32 holds the output bias (ones row lives in h3s tiles)
            wd_t = consts.tile([33, OD], BF, tag="wd")
            nc.scalar.dma_start(out=wd_t[:], in_=wd_d[:])

            # ---- layer 1 for all nodes: h1[m] holds nodes 4m..4m+3 ----
            h1 = []
            for m in range(5):
                psA = psAp.tile([128, BS], FP, tag="psA")
                nc.tensor.matmul(
                    psA[:], wexp_t[:, m * 128:(m + 1) * 128], xT[:],
                    start=True, stop=True,
                )
                h1m = consts.tile([128, BS], BF, name=f"h1_{m}", tag=f"h1_{m}")
                nc.scalar.activation(
                    out=h1m[:], in_=psA[:], func=relu, bias=ba4_t[:], scale=1.0
                )
                h1.append(h1m)

            # ---- layers 2-4 + banded assembly, per batch sub-tile ----
            # g2/g1 rows for node j live in a contiguous free-dim region.
            # c=0 flushes per-node for the first nodes (start the output
            # stream ASAP), then in larger chunks for DMA efficiency.
            for c in range(NCT):
                g2t = g2b[c % 2]
                g1t = g1b[c % 2]
                fft = foutp.tile([128, U + GW], FP, tag="fft")
                g1v = g1t.rearrange("p (r q) -> p r q", q=GW)
                g2v = g2t.rearrange("p (r q) -> p r q", q=GW)
                cs = slice(c * P, (c + 1) * P)
                flush_after = ([0, 1, 2, 3, 4, 6, 8, 11, 14, 19] if c == 0
                               else [4, 9, 14, 19])
                prev_flush = -1
                for j in range(U):
                    mm, k = j // 4, j % 4
                    ps2 = ps2p.tile([64, P], FP, tag="ps2")
                    nc.tensor.matmul(
                        ps2[:], wb4_t[32 * k:32 * k + 32, :],
                        h1[mm][32 * k:32 * k + 32, cs],
                        start=True, stop=True,
                        tile_position=(32 * k, 0),
                    )
                    h2 = h2p.tile([64, P], BF, tag="h2")
                    nc.scalar.activation(
                        out=h2[:], in_=ps2[:], func=relu, bias=bb_t[:], scale=1.0
                    )
                    ps3 = ps3p.tile([32, P], FP, tag="ps3")
                    nc.tensor.matmul(ps3[:], wc_t[:], h2[:], start=True, stop=True)
                    h3 = h3s[j % 4]
                    nc.scalar.activation(
                        out=h3[0:32, :], in_=ps3[:], func=relu, bias=bc_t[:], scale=1.0
                    )
                    ps4 = ps4p.tile([P, OD], FP, tag="ps4")
                    nc.tensor.matmul(ps4[:], h3[:], wd_t[:], start=True, stop=True)

                    ps4g2 = ps4[:, 25:OD].rearrange("p (m t) -> p m t", t=30)
                    for (d0, t0, n) in _g2_segs(j):
                        nc.vector.tensor_copy(
                            out=g2v[:, 6 * j:6 * j + 6, d0:d0 + n],
                            in_=ps4g2[:, :, t0:t0 + n],
                        )
                    for (d0, t0, n) in _g1_segs(j):
                        nc.vector.tensor_copy(
                            out=g1v[:, j, d0:d0 + n], in_=ps4[:, 1 + t0:1 + t0 + n]
                        )
                    nc.vector.tensor_copy(out=fft[:, j:j + 1], in_=ps4[:, 0:1])
                    nc.vector.tensor_copy(
                        out=fft[:, U + 6 * j:U + 6 * j + 6], in_=ps4[:, 19:25]
                    )
                    if j in flush_after:
                        r2 = slice((prev_flush + 1) * 6 * GW, (j + 1) * 6 * GW)
                        r1 = slice((prev_flush + 1) * GW, (j + 1) * GW)
                        nc.sync.dma_start(out=g2_d[cs, r2], in_=g2t[:, r2])
                        nc.sync.dma_start(out=g1_d[cs, r1], in_=g1t[:, r1])
                        prev_flush = j

                nc.sync.dma_start(out=ff_d[cs, :], in_=fft[:])

    nc.compile()
    return nc


_NC_CACHE = None


def _get_nc():
    global _NC_CACHE
    if _NC_CACHE is None:
        _NC_CACHE = _build_bass()
    return _NC_CACHE


def _host_weights(inputs):
    def a(name):
        return np.ascontiguousarray(np.asarray(inputs[name]), dtype=np.float32)

    W1a, W2a = a("W1a"), a("W2a")
    b1a, b2a = a("b1a"), a("b2a")

    def blockdiag(A, B):
        o = np.zeros((A.shape[0] + B.shape[0], A.shape[1] + B.shape[1]), np.float32)
        o[:A.shape[0], :A.shape[1]] = A
        o[A.shape[0]:, A.shape[1]:] = B
        return o

    W_a = np.concatenate([W1a, W2a], axis=1)            # [3, 32]
    b_a = np.concatenate([b1a, b2a])                    # [32]
    W_b = blockdiag(a("W1b"), a("W2b"))                 # [32, 64]
    b_b = np.concatenate([a("b1b"), a("b2b")])          # [64]
    W_c = blockdiag(a("W1c"), a("W2c"))                 # [64, 32]
    b_c = np.concatenate([a("b1c"), a("b2c")])          # [32]
    W_d = blockdiag(a("W1d"), a("W2d"))                 # [32, 205]
    b_d = np.concatenate([a("b1d"), a("b2d")])          # [205]

    # expanded layer-1 stationary matrix: out block j = sum_r wexp[r, 32j:32j+32]*x[:, r]
    wexp = np.zeros((U, U * 32), np.float32)
    for j in range(U):
        wexp[(j - 1) % U, 32 * j:32 * j + 32] = W_a[0]
        wexp[j, 32 * j:32 * j + 32] = W_a[1]
        wexp[(j + 1) % U, 32 * j:32 * j + 32] = W_a[2]

    def bf(arr):
        return np.ascontiguousarray(arr.astype(NPBF))

    return {
        "wexp": bf(wexp),
        "ba4": np.ascontiguousarray(np.tile(b_a, 4)[:, None]),
        "wb4": bf(np.tile(W_b, (4, 1))),
        "bb": np.ascontiguousarray(b_b[:, None]),
        "wc": bf(W_c),
        "bc": np.ascontiguousarray(b_c[:, None]),
        "wd": bf(np.vstack([W_d, b_d[None, :]])),
    }


def make_in_maps(inputs):
    x = np.asarray(inputs["x"], dtype=np.float32)
    assert x.shape == (B_FULL, U), x.shape
    base = _host_weights(inputs)
    return [
        dict(base, xt=np.ascontiguousarray(x[i * BS:(i + 1) * BS].T.astype(NPBF)))
        for i in range(N_CORES)
    ]


def assemble_outputs(results):
    ff = np.concatenate([r["ff"] for r in results], axis=0)
    f1 = np.ascontiguousarray(ff[:, :U]).reshape(B_FULL, U, 1)
    f2 = np.ascontiguousarray(ff[:, U:]).reshape(B_FULL, GW, 1)
    g1 = np.concatenate([r["g1"] for r in results], axis=0).reshape(B_FULL, U, GW)
    g2 = np.concatenate([r["g2"] for r in results], axis=0).reshape(B_FULL, GW, GW)
    return f1, g1, f2, g2


def run_spmd(inputs, **kwargs):
    nc = _get_nc()
    in_maps = make_in_maps(inputs)
    return run_bass_kernel_spmd(nc, in_maps, list(range(N_CORES)), **kwargs)


def kernel(**inputs):
    # Retry once to ride out transient accelerator/runtime hiccups.
    try:
        res = run_spmd(inputs)
    except Exception:
        import time

        time.sleep(3.0)
        res = run_spmd(inputs)
    return assemble_outputs(res.results)
